# revision 58
# baseline (speedup 1.0000x reference)
"""DistanceFromAnswerLoss on 8 Trainium2 NeuronCores.

out = 0.1 * sum_{b,c} mask[b,c] * exp(input[b,c])
  mask[b,c] = |c - t_b| / sqrt(sum_c (c - t_b)^2),  mask = 0 where t_b == 0

Host: rows sorted by t, 512/core, transposed (columns on partitions);
row norms in closed form.  Per core a contiguous 16-block window covers
every t; outside it sign(c - t_b) is constant per 128-column block, so
with m = window center

  sum_{c in agg} |c-t_b| e[c,b] = A~[b] + (m - t_b) * S~[b]
    A~ = sum +-(c-m) e,  S~ = sum +-e   (per-block [+-(c-m), +-1] 2-col
                                         matmuls into a [2, 512] PSUM)

Window blocks: weights fold into exp's argument on the host
(|c-t| e^x = e^{x + ln|c-t|}); their sum rides the A~ PSUM row via a
[1, 0] stationary column.  Final: one scalar_tensor_tensor dots both
PSUM rows with [scale; scale*(m-t)] (accum_out), out = [2, 2] partials.

exp splits across two engines so neither is the wall:
 - 32 aggregate blocks ship as fp8_e4m3 and exp on ACT (dtype-blind
   1 elem/lane/cycle, ~3.7us per 8-slot call, ~15us chain); a warmup
   exp on a preloaded const AP pulls the ~2.7us ACT table load into
   the DMA spin-up dead time; ACT writes e back as fp8 so slot PAIRS
   contract in one fp8 DoubleRow matmul (stationary [Ki, Ko=2, M] with
   16B Ko stride, weights scaled 1/64 into e4m3 range, 64 folded into
   the final scales) — halves the PE runway behind the ACT chain;
 - 16 aggregate + 16 window blocks ship as bf16 and exp on the DVE as
   a Schraudolph bitcast (one 4x-mode tensor_scalar per chunk:
   e^x ~= bitcast_bf16(int16(x * 128/ln2 + 127*128 - CADJ)), CADJ
   calibrated so the sawtooth's weighted mean error is ~0).

Measured ~35.1us (best batch) vs 62.3us baseline; rel err ~1e-3
(fp8 weight quantization dominated).  Schedule notes (from traces):
 - one sync-ring DMA stream, 8KB-per-partition chunks where possible
   (4KB descriptors measured ~300 GB/s vs ~390 GB/s for 8KB); the two
   head chunks are 4KB so the ACT chain starts ~1us earlier, the two
   tail chunks are 4-slot so only ~1.5us of work trails the last byte;
 - 8 dummy matmuls on the first chunk's raw bytes pre-ramp the PE
   p-state (matmuls start at 2.4 GHz instead of ramping 1.2 -> 2.4);
 - PE consumes 8/4-slot units in expected e-readiness order (PE runs
   in program order, so a late exp must not block ready matmuls);
 - PSUM accumulation splits at slot 60: the big group combines
   mid-stream, only a 4-slot group's combine trails the final matmul;
 - measured engine busy: ACT ~15us, DMA ~17-21us (378 GB/s on a good
   run; the 8-core HBM contention drifts run to run), DVE ~7us,
   PE ~14us.
"""

import sys
from contextlib import ExitStack

import numpy as np
import ml_dtypes

sys.path.insert(0, "/opt/trn_rl_repo")

import concourse.bass as bass
import concourse.tile as tile
from concourse import bacc, mybir
from concourse.bass_utils import run_bass_kernel_spmd

B = 4096
C = 8192
N_CORES = 8
ROWS = B // N_CORES          # 512 rows (free dim) per core
NQ = C // 128                # 64 column blocks of 128 (partition dim)
NS = 16                      # window blocks (log-baked weights, contiguous)
NAGG = NQ - NS               # 48 aggregate blocks
COEFF = 0.1

SCHR_SCALE = float(np.float32(128.0 / np.log(2.0)))
CADJ = 7.33                  # sawtooth centering (HW convert rounds)
SCHR_BIAS = float(np.float32(127.0 * 128.0 - CADJ))
LW_CLAMP = -50.0             # ln-weight clamp (e^{x-50} ~ 0)

# Stream plan (6.29 MB total): 32 agg slots ship fp8 (exp on ACT), 16
# agg + 16 window slots ship bf16 (exp = DVE Schraudolph).  Chunks are
# [128 x 8KB] transfers — measured ~390 GB/s vs ~300 GB/s for 4KB
# descriptors — except the first two 4KB ones, split so the ACT chain
# starts ~1.3us earlier.  exp granularity decouples from DMA granularity:
# every exp call covers 8 slots ([128, 4096]).
# CHUNKS: (cid, buffer, slot offset in buffer, slots)
CHUNKS = [
    ("A0a", "x8", 0, 8), ("A0b", "x8", 8, 8), ("D0", "x8", 32, 8),
    ("A1", "x8", 16, 16), ("D1", "x8", 40, 8), ("D2", "xb", 0, 8),
    ("D3a", "xb", 8, 4), ("D3b", "xb", 12, 4),
]
# exp/PE units: unit -> (chunk, col offset in chunk, slots).  The last
# ACT call and the last DVE chunk split into 4-slot units so only 4
# matmuls trail the final exp.
UNITS = {
    "A0a": ("A0a", 0, 8), "A0b": ("A0b", 0, 8), "D0": ("D0", 0, 8),
    "A1a": ("A1", 0, 8), "A1b1": ("A1", 8, 4), "A1b2": ("A1", 12, 4),
    "D1": ("D1", 0, 8), "D2": ("D2", 0, 8),
    "D3a": ("D3a", 0, 4), "D3b": ("D3b", 0, 4),
}
# PE consumes units in expected e-readiness order (PE runs in program
# order, so a late exp must not sit ahead of ready matmuls)
PE_ORDER = ["A0a", "D0", "A0b", "D1", "A1a", "D2", "D3a", "D3b",
            "A1b1", "A1b2"]
QCUT = NQ - 8                        # jA covers slots 0..55
N_PRIME = 12                         # dummy matmuls to pre-ramp PE p-state

F32 = mybir.dt.float32
BF16 = mybir.dt.bfloat16
FP8 = mybir.dt.float8e4
I16 = mybir.dt.int16
Op = mybir.AluOpType
Af = mybir.ActivationFunctionType


def _build() -> bass.Bass:
    nc = bacc.Bacc("TRN2", target_bir_lowering=False, debug=False)
    x8 = nc.declare_dram_parameter("x8", [128, 48, ROWS], FP8, isOutput=False)
    xb = nc.declare_dram_parameter("xb", [128, 16, ROWS], BF16, isOutput=False)
    wv = nc.declare_dram_parameter("wv", [128, 2 * NQ], BF16, isOutput=False)
    # fp8 DoubleRow stationary pairs: [pair-ko (2 per pair), 16-col pad]
    # (the Ko dim must sit 16 bytes apart; only cols 0-1 of each 16 used)
    wv8 = nc.declare_dram_parameter("wv8", [128, 32, 16], FP8, isOutput=False)
    scs = nc.declare_dram_parameter("scs", [2, ROWS], F32, isOutput=False)
    out = nc.declare_dram_parameter("out", [2, 2], F32, isOutput=True)

    with tile.TileContext(nc) as tc, ExitStack() as ctx:
        const_pool = ctx.enter_context(tc.tile_pool(name="const", bufs=1))
        xpool = ctx.enter_context(tc.tile_pool(name="x", bufs=1))
        epool = ctx.enter_context(tc.tile_pool(name="e", bufs=1))
        spool = ctx.enter_context(tc.tile_pool(name="s", bufs=1))
        psum_pool = ctx.enter_context(tc.tile_pool(name="ps", bufs=1, space="PSUM"))

        # ACT table load happens during the DMA spin-up dead time (input
        # is a framework-preloaded const AP: no producer instruction)
        warme = const_pool.tile([128, 1], BF16)
        nc.scalar.activation(
            warme[:], nc.const_aps.tensor(0.0, (128, 1), F32), Af.Exp
        )

        # constants on the scalar HWDGE ring
        wvt = const_pool.tile([128, 2 * NQ], BF16)
        nc.scalar.dma_start(out=wvt[:], in_=wv[:, :])
        wv8t = const_pool.tile([128, 32, 16], FP8)
        nc.scalar.dma_start(out=wv8t[:, :, :], in_=wv8[:, :, :])
        scst = const_pool.tile([2, ROWS], F32)
        nc.scalar.dma_start(out=scst[:], in_=scs[:, :])

        # x stream on the sync ring; exp per 8-slot unit in stream order
        xt = {}
        for cid, buf, s0, nsl in CHUNKS:
            dt_, src_ = (FP8, x8) if buf == "x8" else (BF16, xb)
            t = xpool.tile([128, nsl, ROWS], dt_, name=f"x{cid}")
            nc.sync.dma_start(out=t[:, :, :], in_=src_[:, s0:s0 + nsl, :])
            xt[cid] = t

        et = {}
        for uid, (cid, off, nsl) in UNITS.items():
            xin = xt[cid][:, off:off + nsl, :]
            if uid[0] == "A":
                # fp8 e feeds DoubleRow matmuls (3D: slot pairs on dim 1)
                e = epool.tile([128, nsl, ROWS], FP8, name=f"e{uid}")
                nc.scalar.activation(e[:, :, :], xin, Af.Exp)
                et[uid] = e[:, :, :]
            else:
                # DVE Schraudolph; fp8 agg chunks run 1x, bf16 window 4x
                e = epool.tile([128, nsl, ROWS], I16, name=f"e{uid}")
                nc.vector.tensor_scalar(
                    e[:, :, :], xin, SCHR_SCALE, SCHR_BIAS,
                    op0=Op.mult, op1=Op.add,
                )
                et[uid] = e[:, :, :].bitcast(BF16)

        aspsA = psum_pool.tile([2, ROWS], F32, tag="pasA")
        aspsB = psum_pool.tile([2, ROWS], F32, tag="pasB")
        res = spool.tile([2, 2], F32)

        # PE p-state priming: dummy matmuls on the first chunk's raw bytes
        # (results discarded) so the real matmul stream starts at 2.4 GHz
        prps = psum_pool.tile([1, ROWS // 2], F32, tag="prime")
        pm = xt[CHUNKS[0][0]][:, 0, :].bitcast(BF16)
        for _ in range(N_PRIME):
            nc.tensor.matmul(
                prps[:], wvt[:, 0:1], pm[:, :], start=True, stop=True,
                skip_group_check=True,
            )

        def emit_jA():
            # group A combine runs mid-stream, hidden under the tail
            jA = spool.tile([2, ROWS], F32, name="jA")
            nc.vector.scalar_tensor_tensor(
                jA[:], aspsA[:], 0.0, scst[:],
                op0=Op.add, op1=Op.mult, accum_out=res[:, 0:1],
            )

        q = 0
        pair = 0
        for uid in PE_ORDER:
            nsl = UNITS[uid][2]
            eb = et[uid]
            if uid[0] == "A":
                # fp8 DoubleRow: one matmul per slot pair
                for k in range(0, nsl, 2):
                    ps = aspsA if q < QCUT else aspsB
                    nc.tensor.matmul(
                        ps[:], wv8t[:, 2 * pair:2 * pair + 2, 0:2],
                        eb[:, k:k + 2, :],
                        start=(q == 0 or q == QCUT),
                        stop=(q + 1 == QCUT - 1 or q + 1 == NQ - 1),
                        perf_mode=mybir.MatmulPerfMode.DoubleRow,
                    )
                    if q + 1 == QCUT - 1:
                        emit_jA()
                    pair += 1
                    q += 2
            else:
                for k in range(nsl):
                    ps = aspsA if q < QCUT else aspsB
                    nc.tensor.matmul(
                        ps[:], wvt[:, 2 * q:2 * q + 2],
                        eb[:, k, :],
                        start=(q == 0 or q == QCUT),
                        stop=(q == QCUT - 1 or q == NQ - 1),
                    )
                    if q == QCUT - 1:
                        emit_jA()
                    q += 1

        jB = spool.tile([2, ROWS], F32, name="jB")
        nc.vector.scalar_tensor_tensor(
            jB[:], aspsB[:], 0.0, scst[:], op0=Op.add, op1=Op.mult,
            accum_out=res[:, 1:2],
        )
        nc.scalar.dma_start(out=out[:, :], in_=res[:])

    nc.finalize()
    return nc


_NC = None


def _get_nc() -> bass.Bass:
    global _NC
    if _NC is None:
        _NC = _build()
    return _NC


def _plan(target: np.ndarray):
    """Sort rows by target; per core pick a contiguous 16-block window
    covering all its targets and the agg/window block split."""
    t = np.asarray(target).astype(np.int64).reshape(B)
    order = np.argsort(t, kind="stable")
    plans = []
    for k in range(N_CORES):
        rows = order[k * ROWS:(k + 1) * ROWS]
        tc = t[rows]
        blo, bhi = int(tc.min()) >> 7, int(tc.max()) >> 7
        span = bhi - blo + 1
        assert span <= NS, f"target spread too wide for window: {span} blocks"
        wlo = min(max(blo - (NS - span) // 2, 0), NQ - NS)
        assert wlo <= blo and bhi < wlo + NS
        win = np.arange(wlo, wlo + NS)
        rest = np.array([q for q in range(NQ) if q < wlo or q >= wlo + NS])
        plans.append((rows, tc, win, rest))
    return plans


def _stream_slots(win, rest):
    """Block id per PE slot (PE_ORDER order).

    x8 carries rest[0:32] (units A0a, A0b, A1a, A1b); xb carries
    rest[32:48] (D0, D1) then the 16 window blocks (D2, D3).
    """
    unit_blocks = {
        "A0a": rest[0:8], "A0b": rest[8:16],
        "A1a": rest[16:24], "A1b1": rest[24:28], "A1b2": rest[28:32],
        "D0": rest[32:40], "D1": rest[40:48],
        "D2": win[0:8], "D3a": win[8:12], "D3b": win[12:16],
    }
    slots = []
    for uid in PE_ORDER:
        slots += list(unit_blocks[uid])
    return np.array(slots)


def _pe_unit_per_slot():
    """True per PE slot if it belongs to an A (fp8/DoubleRow) unit."""
    flags = []
    for uid in PE_ORDER:
        flags += [uid[0] == "A"] * UNITS[uid][2]
    return flags


def make_in_maps(input: np.ndarray, target: np.ndarray) -> list[dict]:
    xf = np.asarray(input, dtype=np.float32)
    plans = _plan(target)
    s1 = (C - 1) * C // 2
    s2 = (C - 1) * C * (2 * C - 1) // 6
    in_maps = []
    p128 = np.arange(128, dtype=np.float64)
    for rows, tc, win, rest in plans:
        slots = _stream_slots(win, rest)
        m = float(win[0] * 128 + (NS * 128) / 2.0)
        xr = xf[rows].reshape(ROWS, NQ, 128)

        # fp8 payload: all 48 agg blocks (A units then D0, D1)
        xa = xr[:, rest, :]                          # [ROWS, 48, 128]
        x8 = np.ascontiguousarray(xa.transpose(2, 1, 0)).astype(
            ml_dtypes.float8_e4m3
        )                                            # [128, 32, ROWS]

        # bf16 payload: the 16 window blocks with ln-weights baked in
        xd = xr[:, win, :].transpose(2, 1, 0).astype(np.float64)
        cw = win[None, :] * 128 + p128[:, None]       # [128, NS]
        dist = np.abs(cw[:, :, None] - tc[None, None, :].astype(np.float64))
        lw = np.log(np.maximum(dist, 1e-30))
        np.maximum(lw, LW_CLAMP, out=lw)
        xd += lw
        xbp = np.ascontiguousarray(xd).astype(ml_dtypes.bfloat16)

        # stationary columns in PE-slot order, scaled by 1/64 so the fp8
        # DoubleRow weights stay inside e4m3 range (64 folded into scs)
        is_win = np.isin(slots, win)
        sgn = np.where(slots * 128 > win[-1] * 128, 1.0, -1.0)
        cs = slots[None, :] * 128 + p128[:, None] - m   # [128, 64]
        wvc = np.zeros((128, 2 * NQ), dtype=np.float32)
        wvc[:, 0::2] = np.where(is_win[None, :], 1.0, cs * sgn[None, :]) / 64
        wvc[:, 1::2] = np.where(is_win[None, :], 0.0, sgn[None, :]) / 64

        # fp8 DoubleRow pairs: A-unit slots in PE order, two blocks per
        # matmul; wv8[p, 2i+j, m] = weight of pair i's member j, PSUM row m
        a_q = [qq for qq, uid_w in enumerate(_pe_unit_per_slot()) if uid_w]
        wv8 = np.zeros((128, 32, 16), dtype=np.float32)
        for i in range(16):
            for j in range(2):
                qq = a_q[2 * i + j]
                wv8[:, 2 * i + j, 0] = wvc[:, 2 * qq]
                wv8[:, 2 * i + j, 1] = wvc[:, 2 * qq + 1]

        norm = np.sqrt(C * tc.astype(np.float64) ** 2 - 2.0 * tc * s1 + s2)
        sc64 = COEFF / np.maximum(norm, 1e-12) * (tc != 0) * 64.0
        scs = np.stack([sc64, sc64 * (m - tc.astype(np.float64))])
        in_maps.append({
            "x8": x8,
            "xb": xbp,
            "wv": wvc.astype(ml_dtypes.bfloat16),
            "wv8": wv8.astype(ml_dtypes.float8_e4m3),
            "scs": np.ascontiguousarray(scs.astype(np.float32)),
        })
    return in_maps


def run(input: np.ndarray, target: np.ndarray, trace: bool = False, tmpdir=None):
    nc = _get_nc()
    in_maps = make_in_maps(input, target)
    res = run_bass_kernel_spmd(
        nc, in_maps, list(range(N_CORES)), trace=trace, tmpdir=tmpdir
    )
    total = np.float32(0.0)
    for r in res.results:
        total += np.float32(r["out"].reshape(-1).sum())
    return np.asarray(total, dtype=np.float32), res


def kernel(input: np.ndarray, target: np.ndarray) -> np.ndarray:
    out, _ = run(input, target)
    return out


# revision 59
# speedup vs baseline: 1.0737x; 1.0737x over previous
"""DistanceFromAnswerLoss on 8 Trainium2 NeuronCores.

out = 0.1 * sum_{b,c} mask[b,c] * exp(input[b,c])
  mask[b,c] = |c - t_b| / sqrt(sum_c (c - t_b)^2),  mask = 0 where t_b == 0

Host: rows sorted by t, 512/core, transposed (columns on partitions);
row norms in closed form.  Per core a contiguous 16-block window covers
every t; outside it sign(c - t_b) is constant per 128-column block, so
with m = window center

  sum_{c in agg} |c-t_b| e[c,b] = A~[b] + (m - t_b) * S~[b]
    A~ = sum +-(c-m) e,  S~ = sum +-e   (per-block [+-(c-m), +-1] 2-col
                                         matmuls into a [2, 512] PSUM)

Window blocks: weights fold into exp's argument on the host
(|c-t| e^x = e^{x + ln|c-t|}); their sum rides the A~ PSUM row via a
[1, 0] stationary column.  Final: one scalar_tensor_tensor dots both
PSUM rows with [scale; scale*(m-t)] (accum_out), out = [2, 2] partials.

exp splits across two engines so neither is the wall:
 - 32 aggregate blocks ship as fp8_e4m3 and exp on ACT (dtype-blind
   1 elem/lane/cycle, ~3.7us per 8-slot call, ~15us chain); a warmup
   exp on a preloaded const AP pulls the ~2.7us ACT table load into
   the DMA spin-up dead time; ACT writes e back as fp8 so slot PAIRS
   contract in one fp8 DoubleRow matmul (stationary [Ki, Ko=2, M] with
   16B Ko stride, weights scaled 1/64 into e4m3 range, 64 folded into
   the final scales) — halves the PE runway behind the ACT chain;
 - 16 aggregate + 16 window blocks ship as bf16 and exp on the DVE as
   a Schraudolph bitcast (one 4x-mode tensor_scalar per chunk:
   e^x ~= bitcast_bf16(int16(x * 128/ln2 + 127*128 - CADJ)), CADJ
   calibrated so the sawtooth's weighted mean error is ~0).

Measured ~35.1us (best batch) vs 62.3us baseline; rel err ~1e-3
(fp8 weight quantization dominated).  Schedule notes (from traces):
 - one sync-ring DMA stream, 8KB-per-partition chunks where possible
   (4KB descriptors measured ~300 GB/s vs ~390 GB/s for 8KB); the two
   head chunks are 4KB so the ACT chain starts ~1us earlier, the two
   tail chunks are 4-slot so only ~1.5us of work trails the last byte;
 - 8 dummy matmuls on the first chunk's raw bytes pre-ramp the PE
   p-state (matmuls start at 2.4 GHz instead of ramping 1.2 -> 2.4);
 - PE consumes 8/4-slot units in expected e-readiness order (PE runs
   in program order, so a late exp must not block ready matmuls);
 - PSUM accumulation splits at slot 60: the big group combines
   mid-stream, only a 4-slot group's combine trails the final matmul;
 - measured engine busy: ACT ~15us, DMA ~17-21us (378 GB/s on a good
   run; the 8-core HBM contention drifts run to run), DVE ~7us,
   PE ~14us.
"""

import sys
from contextlib import ExitStack

import numpy as np
import ml_dtypes

sys.path.insert(0, "/opt/trn_rl_repo")

import concourse.bass as bass
import concourse.tile as tile
from concourse import bacc, mybir
from concourse.bass_utils import run_bass_kernel_spmd

B = 4096
C = 8192
N_CORES = 8
ROWS = B // N_CORES          # 512 rows (free dim) per core
NQ = C // 128                # 64 column blocks of 128 (partition dim)
NS = 16                      # window blocks (log-baked weights, contiguous)
NAGG = NQ - NS               # 48 aggregate blocks
COEFF = 0.1

SCHR_SCALE = float(np.float32(128.0 / np.log(2.0)))
CADJ = 7.33                  # sawtooth centering (HW convert rounds)
SCHR_BIAS = float(np.float32(127.0 * 128.0 - CADJ))
LW_CLAMP = -50.0             # ln-weight clamp (e^{x-50} ~ 0)

# Stream plan (6.29 MB total): 32 agg slots ship fp8 (exp on ACT), 16
# agg + 16 window slots ship bf16 (exp = DVE Schraudolph).  Chunks are
# [128 x 8KB] transfers — measured ~390 GB/s vs ~300 GB/s for 4KB
# descriptors — except the first two 4KB ones, split so the ACT chain
# starts ~1.3us earlier.  exp granularity decouples from DMA granularity:
# every exp call covers 8 slots ([128, 4096]).
# CHUNKS: (cid, buffer, slot offset in buffer, slots)
CHUNKS = [
    ("A0a", "x8", 0, 8), ("A0b", "x8", 8, 8), ("D0", "xb", 0, 8),
    ("A1", "x8", 16, 16), ("D1", "xb", 8, 8), ("D2", "xb", 16, 8),
    ("D3a", "xb", 24, 4), ("D3b", "xb", 28, 4),
]
# exp/PE units: unit -> (chunk, col offset in chunk, slots).  The last
# ACT call and the last DVE chunk split into 4-slot units so only 4
# matmuls trail the final exp.
UNITS = {
    "A0a": ("A0a", 0, 8), "A0b": ("A0b", 0, 8), "D0": ("D0", 0, 8),
    "A1a": ("A1", 0, 8), "A1b1": ("A1", 8, 4), "A1b2": ("A1", 12, 4),
    "D1": ("D1", 0, 8), "D2": ("D2", 0, 8),
    "D3a": ("D3a", 0, 4), "D3b": ("D3b", 0, 4),
}
# PE consumes units in expected e-readiness order (PE runs in program
# order, so a late exp must not sit ahead of ready matmuls)
PE_ORDER = ["A0a", "D0", "A0b", "D1", "A1a", "D2", "A1b1", "D3a",
            "A1b2", "D3b"]
QCUT = NQ - 4                        # jA covers slots 0..59
N_PRIME = 12                         # dummy matmuls to pre-ramp PE p-state

F32 = mybir.dt.float32
BF16 = mybir.dt.bfloat16
FP8 = mybir.dt.float8e4
I16 = mybir.dt.int16
Op = mybir.AluOpType
Af = mybir.ActivationFunctionType


def _build() -> bass.Bass:
    nc = bacc.Bacc("TRN2", target_bir_lowering=False, debug=False)
    x8 = nc.declare_dram_parameter("x8", [128, 32, ROWS], FP8, isOutput=False)
    xb = nc.declare_dram_parameter("xb", [128, 32 * ROWS], BF16, isOutput=False)
    wv = nc.declare_dram_parameter("wv", [128, 2 * NQ], BF16, isOutput=False)
    # fp8 DoubleRow stationary pairs: [pair-ko (2 per pair), 16-col pad]
    # (the Ko dim must sit 16 bytes apart; only cols 0-1 of each 16 used)
    wv8 = nc.declare_dram_parameter("wv8", [128, 32, 16], FP8, isOutput=False)
    scs = nc.declare_dram_parameter("scs", [2, ROWS], F32, isOutput=False)
    out = nc.declare_dram_parameter("out", [2, 2], F32, isOutput=True)

    with tile.TileContext(nc) as tc, ExitStack() as ctx:
        const_pool = ctx.enter_context(tc.tile_pool(name="const", bufs=1))
        xpool = ctx.enter_context(tc.tile_pool(name="x", bufs=1))
        epool = ctx.enter_context(tc.tile_pool(name="e", bufs=1))
        spool = ctx.enter_context(tc.tile_pool(name="s", bufs=1))
        psum_pool = ctx.enter_context(tc.tile_pool(name="ps", bufs=1, space="PSUM"))

        # ACT table load happens during the DMA spin-up dead time (input
        # is a framework-preloaded const AP: no producer instruction)
        warme = const_pool.tile([128, 1], BF16)
        nc.scalar.activation(
            warme[:], nc.const_aps.tensor(0.0, (128, 1), F32), Af.Exp
        )

        # constants on the scalar HWDGE ring
        wvt = const_pool.tile([128, 2 * NQ], BF16)
        nc.scalar.dma_start(out=wvt[:], in_=wv[:, :])
        wv8t = const_pool.tile([128, 32, 16], FP8)
        nc.scalar.dma_start(out=wv8t[:, :, :], in_=wv8[:, :, :])
        scst = const_pool.tile([2, ROWS], F32)
        nc.scalar.dma_start(out=scst[:], in_=scs[:, :])

        # x stream on the sync ring; exp per 8-slot unit in stream order
        xt = {}
        for cid, buf, s0, nsl in CHUNKS:
            if buf == "x8":
                t = xpool.tile([128, nsl, ROWS], FP8, name=f"x{cid}")
                nc.sync.dma_start(out=t[:, :, :], in_=x8[:, s0:s0 + nsl, :])
            else:
                t = xpool.tile([128, nsl * ROWS], BF16, name=f"x{cid}")
                nc.sync.dma_start(
                    out=t[:], in_=xb[:, s0 * ROWS:(s0 + nsl) * ROWS]
                )
            xt[cid] = t

        et = {}
        for uid, (cid, off, nsl) in UNITS.items():
            if uid[0] == "A":
                # fp8 e feeds DoubleRow matmuls (3D: slot pairs on dim 1)
                xin = xt[cid][:, off:off + nsl, :]
                e = epool.tile([128, nsl, ROWS], FP8, name=f"e{uid}")
                nc.scalar.activation(e[:, :, :], xin, Af.Exp)
                et[uid] = e[:, :, :]
            else:
                xin = xt[cid][:, off * ROWS:(off + nsl) * ROWS]
                e = epool.tile([128, nsl * ROWS], I16, name=f"e{uid}")
                nc.vector.tensor_scalar(
                    e[:], xin, SCHR_SCALE, SCHR_BIAS, op0=Op.mult, op1=Op.add
                )
                et[uid] = e[:].bitcast(BF16)

        aspsA = psum_pool.tile([2, ROWS], F32, tag="pasA")
        aspsB = psum_pool.tile([2, ROWS], F32, tag="pasB")
        res = spool.tile([2, 2], F32)

        # PE p-state priming: dummy matmuls on the first chunk's raw bytes
        # (results discarded) so the real matmul stream starts at 2.4 GHz
        prps = psum_pool.tile([1, ROWS // 2], F32, tag="prime")
        pm = xt[CHUNKS[0][0]][:, 0, :].bitcast(BF16)
        for _ in range(N_PRIME):
            nc.tensor.matmul(
                prps[:], wvt[:, 0:1], pm[:, :], start=True, stop=True,
                skip_group_check=True,
            )

        def emit_jA():
            # group A combine runs mid-stream, hidden under the tail
            jA = spool.tile([2, ROWS], F32, name="jA")
            nc.vector.scalar_tensor_tensor(
                jA[:], aspsA[:], 0.0, scst[:],
                op0=Op.add, op1=Op.mult, accum_out=res[:, 0:1],
            )

        q = 0
        pair = 0
        for uid in PE_ORDER:
            nsl = UNITS[uid][2]
            eb = et[uid]
            if uid[0] == "A":
                # fp8 DoubleRow: one matmul per slot pair
                for k in range(0, nsl, 2):
                    ps = aspsA if q < QCUT else aspsB
                    nc.tensor.matmul(
                        ps[:], wv8t[:, 2 * pair:2 * pair + 2, 0:2],
                        eb[:, k:k + 2, :],
                        start=(q == 0 or q == QCUT),
                        stop=(q + 1 == QCUT - 1 or q + 1 == NQ - 1),
                        perf_mode=mybir.MatmulPerfMode.DoubleRow,
                    )
                    if q + 1 == QCUT - 1:
                        emit_jA()
                    pair += 1
                    q += 2
            else:
                for k in range(nsl):
                    ps = aspsA if q < QCUT else aspsB
                    nc.tensor.matmul(
                        ps[:], wvt[:, 2 * q:2 * q + 2],
                        eb[:, k * ROWS:(k + 1) * ROWS],
                        start=(q == 0 or q == QCUT),
                        stop=(q == QCUT - 1 or q == NQ - 1),
                    )
                    if q == QCUT - 1:
                        emit_jA()
                    q += 1

        jB = spool.tile([2, ROWS], F32, name="jB")
        nc.vector.scalar_tensor_tensor(
            jB[:], aspsB[:], 0.0, scst[:], op0=Op.add, op1=Op.mult,
            accum_out=res[:, 1:2],
        )
        nc.scalar.dma_start(out=out[:, :], in_=res[:])

    nc.finalize()
    return nc


_NC = None


def _get_nc() -> bass.Bass:
    global _NC
    if _NC is None:
        _NC = _build()
    return _NC


def _plan(target: np.ndarray):
    """Sort rows by target; per core pick a contiguous 16-block window
    covering all its targets and the agg/window block split."""
    t = np.asarray(target).astype(np.int64).reshape(B)
    order = np.argsort(t, kind="stable")
    plans = []
    for k in range(N_CORES):
        rows = order[k * ROWS:(k + 1) * ROWS]
        tc = t[rows]
        blo, bhi = int(tc.min()) >> 7, int(tc.max()) >> 7
        span = bhi - blo + 1
        assert span <= NS, f"target spread too wide for window: {span} blocks"
        wlo = min(max(blo - (NS - span) // 2, 0), NQ - NS)
        assert wlo <= blo and bhi < wlo + NS
        win = np.arange(wlo, wlo + NS)
        rest = np.array([q for q in range(NQ) if q < wlo or q >= wlo + NS])
        plans.append((rows, tc, win, rest))
    return plans


def _stream_slots(win, rest):
    """Block id per PE slot (PE_ORDER order).

    x8 carries rest[0:32] (units A0a, A0b, A1a, A1b); xb carries
    rest[32:48] (D0, D1) then the 16 window blocks (D2, D3).
    """
    unit_blocks = {
        "A0a": rest[0:8], "A0b": rest[8:16],
        "A1a": rest[16:24], "A1b1": rest[24:28], "A1b2": rest[28:32],
        "D0": rest[32:40], "D1": rest[40:48],
        "D2": win[0:8], "D3a": win[8:12], "D3b": win[12:16],
    }
    slots = []
    for uid in PE_ORDER:
        slots += list(unit_blocks[uid])
    return np.array(slots)


def _pe_unit_per_slot():
    """True per PE slot if it belongs to an A (fp8/DoubleRow) unit."""
    flags = []
    for uid in PE_ORDER:
        flags += [uid[0] == "A"] * UNITS[uid][2]
    return flags


def make_in_maps(input: np.ndarray, target: np.ndarray) -> list[dict]:
    xf = np.asarray(input, dtype=np.float32)
    plans = _plan(target)
    s1 = (C - 1) * C // 2
    s2 = (C - 1) * C * (2 * C - 1) // 6
    in_maps = []
    p128 = np.arange(128, dtype=np.float64)
    for rows, tc, win, rest in plans:
        slots = _stream_slots(win, rest)
        m = float(win[0] * 128 + (NS * 128) / 2.0)
        xr = xf[rows].reshape(ROWS, NQ, 128)

        # fp8 payload: agg blocks rest[0:32]
        xa = xr[:, rest[:32], :]                     # [ROWS, 32, 128]
        x8 = np.ascontiguousarray(xa.transpose(2, 1, 0)).astype(
            ml_dtypes.float8_e4m3
        )                                            # [128, 32, ROWS]

        # bf16 payload: agg blocks rest[32:48], then the 16 window blocks
        # with ln-weights baked in
        d_blocks = np.concatenate([rest[32:], win])
        xd = xr[:, d_blocks, :].transpose(2, 1, 0).astype(np.float64)
        cw = win[None, :] * 128 + p128[:, None]       # [128, NS]
        dist = np.abs(cw[:, :, None] - tc[None, None, :].astype(np.float64))
        lw = np.log(np.maximum(dist, 1e-30))
        np.maximum(lw, LW_CLAMP, out=lw)
        xd[:, 16:, :] += lw
        xbp = np.ascontiguousarray(xd).reshape(
            128, 32 * ROWS
        ).astype(ml_dtypes.bfloat16)

        # stationary columns in PE-slot order, scaled by 1/64 so the fp8
        # DoubleRow weights stay inside e4m3 range (64 folded into scs)
        is_win = np.isin(slots, win)
        sgn = np.where(slots * 128 > win[-1] * 128, 1.0, -1.0)
        cs = slots[None, :] * 128 + p128[:, None] - m   # [128, 64]
        wvc = np.zeros((128, 2 * NQ), dtype=np.float32)
        wvc[:, 0::2] = np.where(is_win[None, :], 1.0, cs * sgn[None, :]) / 64
        wvc[:, 1::2] = np.where(is_win[None, :], 0.0, sgn[None, :]) / 64

        # fp8 DoubleRow pairs: A-unit slots in PE order, two blocks per
        # matmul; wv8[p, 2i+j, m] = weight of pair i's member j, PSUM row m
        a_q = [qq for qq, uid_w in enumerate(_pe_unit_per_slot()) if uid_w]
        wv8 = np.zeros((128, 32, 16), dtype=np.float32)
        for i in range(16):
            for j in range(2):
                qq = a_q[2 * i + j]
                wv8[:, 2 * i + j, 0] = wvc[:, 2 * qq]
                wv8[:, 2 * i + j, 1] = wvc[:, 2 * qq + 1]

        norm = np.sqrt(C * tc.astype(np.float64) ** 2 - 2.0 * tc * s1 + s2)
        sc64 = COEFF / np.maximum(norm, 1e-12) * (tc != 0) * 64.0
        scs = np.stack([sc64, sc64 * (m - tc.astype(np.float64))])
        in_maps.append({
            "x8": x8,
            "xb": xbp,
            "wv": wvc.astype(ml_dtypes.bfloat16),
            "wv8": wv8.astype(ml_dtypes.float8_e4m3),
            "scs": np.ascontiguousarray(scs.astype(np.float32)),
        })
    return in_maps


def run(input: np.ndarray, target: np.ndarray, trace: bool = False, tmpdir=None):
    nc = _get_nc()
    in_maps = make_in_maps(input, target)
    res = run_bass_kernel_spmd(
        nc, in_maps, list(range(N_CORES)), trace=trace, tmpdir=tmpdir
    )
    total = np.float32(0.0)
    for r in res.results:
        total += np.float32(r["out"].reshape(-1).sum())
    return np.asarray(total, dtype=np.float32), res


def kernel(input: np.ndarray, target: np.ndarray) -> np.ndarray:
    out, _ = run(input, target)
    return out
